# revision 15
# baseline (speedup 1.0000x reference)
"""Trainium2 Bass kernel for IntegralTransform GNN message passing.

Strategy (dst-sharded, 8 cores), v4:
  - Node space padded to 50176 = 8 * 49 * 128. Core c owns nodes
    [c*6272, (c+1)*6272) = 49 buckets of 128 nodes.
  - Host bins edges by dst bucket (stable sort), gathers pos[src]|pos[dst]
    and x[src], pads to K chunks of 128 edges per bucket, and ALSO
    precomputes the per-chunk one-hot dst matrices (bf16) so no engine
    spends time on iota/compare — they stream in over the idle DMA path.
  - PE array packing: L1/L2 run as two concurrent 64x64 tiles
    (T0 = partitions 0-63, T10 = 64-127; contraction zero-padded to 64),
    L3 and the one-hot scatter run as two concurrent 64x128 row tiles
    (T0/T8), which also hides LDWEIGHTS behind the other tile's matmul.
  - Per chunk, L3 makes hp = h2 @ Wo edge-major ([128e, 256(o,i)]); DVE
    multiplies hp (PSUM, 4 chunks per instruction) by xs broadcast along
    o into bf16 prod tiles; one-hot matmuls scatter prod into two
    node-major PSUM accumulators (one per edge half).
  - Bucket drain: DVE group-reduces both accs over i -> [128, 2, 16].
  - Host sums the halves, adds the exact bias term via a bincount
    xs-aggregation: out = msum + xs_agg @ bo. No collectives.
"""

import numpy as np
import ml_dtypes

N_POINTS = 50000
N_PAD = 50176          # 8 * 49 * 128
N_CORES = 8
BUCKET = 128           # nodes per bucket
B_PER_CORE = 49
N_BUCKETS = N_PAD // BUCKET   # 392
CORE_NODES = B_PER_CORE * BUCKET  # 6272
IN_CH = 16
OUT_CH = 16
HID = 64
POS = 3

BF16 = ml_dtypes.bfloat16

_PROGRAM_CACHE = {}


def _geom(K):
    """Pair-tile geometry for S = K*128 edge slots per bucket."""
    S = K * 128
    n_full = S // 1024           # full pair tiles (1024 slots as [128, 512])
    tail = S - n_full * 1024     # leftover slots, multiple of 128, < 1024
    CB = n_full * 512 + tail     # pos-enc columns per bucket (tail unstacked)
    return S, n_full, tail, CB


def _build_program(K):
    import concourse.bacc as bacc
    import concourse.tile as tile
    import concourse.mybir as mybir

    f32 = mybir.dt.float32
    bf16 = mybir.dt.bfloat16
    S, n_full, tail, CB = _geom(K)

    nc = bacc.Bacc("TRN2", target_bir_lowering=False, debug=False)

    PT = nc.dram_tensor("PT", [128, B_PER_CORE * CB], bf16, kind="ExternalInput")
    XSB = nc.dram_tensor("XSB", [B_PER_CORE, 128, K * 16], bf16, kind="ExternalInput")
    OH = nc.dram_tensor("OH", [B_PER_CORE, 128, K * 128], bf16, kind="ExternalInput")
    W1S = nc.dram_tensor("W1S", [128, HID], bf16, kind="ExternalInput")
    B1S = nc.dram_tensor("B1S", [128, 1], f32, kind="ExternalInput")
    WH2 = nc.dram_tensor("WH2", [128, HID], bf16, kind="ExternalInput")
    BH2 = nc.dram_tensor("BH2", [128, 1], f32, kind="ExternalInput")
    WO2 = nc.dram_tensor("WO2", [128, 256], bf16, kind="ExternalInput")
    OUT = nc.dram_tensor("OUT", [128, B_PER_CORE * 512], bf16, kind="ExternalOutput")

    Gelu = mybir.ActivationFunctionType.Gelu
    Copy = mybir.ActivationFunctionType.Copy
    MUL = mybir.AluOpType.mult
    ADD = mybir.AluOpType.add
    X = mybir.AxisListType.X

    # chunk -> (partition row base, column range) inside its bucket's h2 tiles
    def chunk_loc(c):
        s0 = c * 128
        if s0 < n_full * 1024:
            t = s0 // 1024
            w = s0 % 1024
            if w < 512:
                return 0, t * 512 + w
            return 64, t * 512 + (w - 512)
        return 0, n_full * 512 + (s0 - n_full * 1024)

    batches = []
    c = 0
    while c < K:
        n = min(4, K - c)
        batches.append((c, n))
        c += n

    with tile.TileContext(nc) as tc:
        with tc.tile_pool(name="const", bufs=1) as cp, \
             tc.tile_pool(name="io", bufs=3) as io, \
             tc.tile_pool(name="h1p", bufs=3) as h1p, \
             tc.tile_pool(name="h2p", bufs=6) as h2p, \
             tc.tile_pool(name="wk", bufs=4) as wk, \
             tc.tile_pool(name="psL", bufs=2, space="PSUM") as psL, \
             tc.tile_pool(name="psH", bufs=2, space="PSUM") as psH, \
             tc.tile_pool(name="psAcc", bufs=1, space="PSUM") as psAcc:

            # --- constants ---
            w1s_t = cp.tile([128, HID], bf16)
            nc.sync.dma_start(out=w1s_t[:], in_=W1S[:])
            b1s_t = cp.tile([128, 1], f32)
            nc.sync.dma_start(out=b1s_t[:], in_=B1S[:])
            wh2_t = cp.tile([128, HID], bf16)
            nc.sync.dma_start(out=wh2_t[:], in_=WH2[:])
            bh2_t = cp.tile([128, 1], f32)
            nc.sync.dma_start(out=bh2_t[:], in_=BH2[:])
            wo2_t = cp.tile([128, 256], bf16)
            nc.sync.dma_start(out=wo2_t[:], in_=WO2[:])
            fin_sb = cp.tile([128, B_PER_CORE, 512], bf16)

            h2_tiles = {}

            def stage1(b):
                """DMA in + L1/L2 MLP (64x64 packed pairs)."""
                pt_t = io.tile([128, CB], bf16, tag="pt")
                nc.sync.dma_start(out=pt_t[:], in_=PT[:, b * CB:(b + 1) * CB])
                xsb_t = io.tile([128, K, 16], bf16, tag="xsb")
                nc.sync.dma_start(out=xsb_t[:], in_=XSB[b])
                oh_t = io.tile([128, K, 128], bf16, tag="oh")
                nc.sync.dma_start(out=oh_t[:], in_=OH[b])

                tiles = []
                for t in range(n_full):
                    c0 = t * 512
                    p1 = psL.tile([128, 512], f32, tag="psl")
                    nc.tensor.matmul(p1[0:64, :], lhsT=w1s_t[0:64, :],
                                     rhs=pt_t[0:64, c0:c0 + 512],
                                     start=True, stop=True)
                    nc.tensor.matmul(p1[64:128, :], lhsT=w1s_t[64:128, :],
                                     rhs=pt_t[64:128, c0:c0 + 512],
                                     start=True, stop=True)
                    h1_t = h1p.tile([128, 512], bf16, tag="h1")
                    nc.scalar.activation(h1_t[:], p1[:], Gelu,
                                         bias=b1s_t[:], scale=1.0)
                    p2 = psL.tile([128, 512], f32, tag="psl")
                    nc.tensor.matmul(p2[0:64, :], lhsT=wh2_t[0:64, :],
                                     rhs=h1_t[0:64, :], start=True, stop=True)
                    nc.tensor.matmul(p2[64:128, :], lhsT=wh2_t[64:128, :],
                                     rhs=h1_t[64:128, :], start=True, stop=True)
                    h2_t = h2p.tile([128, 512], bf16, tag="h2")
                    nc.scalar.activation(h2_t[:], p2[:], Gelu,
                                         bias=bh2_t[:], scale=1.0)
                    tiles.append(h2_t)
                if tail:
                    h2tl = h2p.tile([64, tail], bf16, tag="h2t")
                    for w0 in range(0, tail, 512):
                        w = min(512, tail - w0)
                        c0 = n_full * 512 + w0
                        p1 = psL.tile([128, 512], f32, tag="psl")
                        nc.tensor.matmul(p1[0:64, 0:w], lhsT=w1s_t[0:64, :],
                                         rhs=pt_t[0:64, c0:c0 + w],
                                         start=True, stop=True)
                        h1tl = h1p.tile([64, 512], bf16, tag="h1t")
                        nc.scalar.activation(h1tl[:, 0:w], p1[0:64, 0:w], Gelu,
                                             bias=b1s_t[0:64], scale=1.0)
                        p2 = psL.tile([128, 512], f32, tag="psl")
                        nc.tensor.matmul(p2[0:64, 0:w], lhsT=wh2_t[0:64, :],
                                         rhs=h1tl[0:64, 0:w], start=True, stop=True)
                        nc.scalar.activation(h2tl[:, w0:w0 + w], p2[0:64, 0:w],
                                             Gelu, bias=bh2_t[0:64], scale=1.0)
                    tiles.append(h2tl)
                h2_tiles[b] = (tiles, xsb_t, oh_t)

            def emit_l3(c, hp_t, j, tiles):
                row, col = chunk_loc(c)
                if row == 0 and col >= n_full * 512:
                    src = tiles[n_full]   # tail tile
                    lhsT = src[0:64, col - n_full * 512:
                               col - n_full * 512 + 128]
                    rhs = wo2_t[0:64, :]
                else:
                    src = tiles[col // 512]
                    cc = col % 512
                    lhsT = src[row:row + 64, cc:cc + 128]
                    rhs = wo2_t[row:row + 64, :]
                nc.tensor.matmul(hp_t[:, j * 256:(j + 1) * 256],
                                 lhsT=lhsT, rhs=rhs, start=True, stop=True)

            def stage2(b):
                """L3 + einsum + scatter (64x128 row tiles T0/T8)."""
                tiles, xsb_t, oh_t = h2_tiles.pop(b)

                acc2 = psAcc.tile([128, 2, 512], f32, tag="acc")
                pend = []
                bi = 0
                while bi < len(batches):
                    # Process two consecutive batches together, interleaving
                    # their L3 matmuls: adjacent batches cover the two PE
                    # half-regions (T0/T8 row tiles) and sit in different
                    # PSUM banks, so interleaved matmuls run concurrently.
                    grp = batches[bi:bi + 2]
                    bi += len(grp)
                    hps = []
                    for _g in range(len(grp)):
                        hp_t = psH.tile([128, 4 * 256], f32, tag="hp",
                                        name=f"hp_{bi}_{_g}")
                        hps.append(hp_t)
                    maxn = max(n for (_, n) in grp)
                    for j in range(maxn):
                        for g, (c0, n) in enumerate(grp):
                            if j < n:
                                emit_l3(c0 + j, hps[g], j, tiles)
                    for g, (c0, n) in enumerate(grp):
                        # prod tile [128, c, 16, 16] bf16 = hp * xs(bcast o)
                        scat4 = wk.tile([128, 4, 16, 16], bf16, tag="scat")
                        nc.vector.tensor_tensor(
                            out=scat4[:, 0:n, :, :],
                            in0=hps[g][:, 0:n * 256].rearrange(
                                "p (c o i) -> p c o i", c=n, o=16, i=16),
                            in1=xsb_t[:, c0:c0 + n, :].unsqueeze(2)
                                .to_broadcast([128, n, 16, 16]),
                            op=MUL)
                        pend.append((scat4, c0, n))
                        if len(pend) == 3:
                            _scatter(pend.pop(0), oh_t, acc2)
                while pend:
                    _scatter(pend.pop(0), oh_t, acc2)

                # bucket drain: raw acc halves to SBUF bf16 on ACT; the
                # host does the i-reduction (sum over halves and i).
                nc.scalar.activation(fin_sb[:, b, :], acc2[:, :, 0:256], Copy)

            def _scatter(item, oh_t, acc2):
                scat4, c0, n = item
                for j in range(n):
                    c = c0 + j
                    st = (c == 0)
                    sp = (c == K - 1)
                    nc.tensor.matmul(acc2[:, 0, 0:256],
                                     lhsT=oh_t[0:64, c, :],
                                     rhs=scat4[0:64, j, :, :],
                                     start=st, stop=sp)
                    nc.tensor.matmul(acc2[:, 1, 0:256],
                                     lhsT=oh_t[64:128, c, :],
                                     rhs=scat4[64:128, j, :, :],
                                     start=st, stop=sp)

            stage1(0)
            for b in range(B_PER_CORE):
                if b + 1 < B_PER_CORE:
                    stage1(b + 1)
                stage2(b)

            nc.sync.dma_start(out=OUT[:], in_=fin_sb[:])

    nc.compile()
    return nc


def _host_prep(x, pos, edge_index, W1, b1, Wh, bh, Wo, bo):
    """Bin edges by dst bucket, gather, pad; build per-core input maps."""
    x_flat = np.ascontiguousarray(x.reshape(-1, IN_CH).astype(np.float32))
    pos = np.ascontiguousarray(pos.astype(np.float32))
    src = np.asarray(edge_index[0], dtype=np.int64)
    dst = np.asarray(edge_index[1], dtype=np.int64)
    E = src.shape[0]

    bucket = (dst >> 7).astype(np.int32)          # 0..390
    order = np.argsort(bucket, kind="stable")
    sb = bucket[order]
    cnt = np.bincount(bucket, minlength=N_BUCKETS)
    K = int(np.max((cnt + 127) // 128))
    S, n_full, tail, CB = _geom(K)

    starts = np.zeros(N_BUCKETS, dtype=np.int64)
    starts[1:] = np.cumsum(cnt)[:-1]
    rank = np.arange(E, dtype=np.int64) - starts[sb]
    slot = sb.astype(np.int64) * S + rank          # global slot id

    e_src = src[order]
    e_dst = dst[order]

    # pos-enc stream in pair-stacked layout
    sl = slot % S
    in_full = sl < n_full * 1024
    w = sl % 1024
    col_full = (sl // 1024) * 512 + np.where(w < 512, w, w - 512)
    grp_full = (w >= 512).astype(np.int64)
    col_tail = n_full * 512 + (sl - n_full * 1024)
    colb = np.where(in_full, col_full, col_tail)
    grp = np.where(in_full, grp_full, 0)
    gcol = (slot // S) * CB + colb

    PT_full = np.zeros((128, N_BUCKETS * CB), dtype=np.float32)
    m0 = grp == 0
    m1 = ~m0
    for d in range(POS):
        PT_full[0 + d, gcol[m0]] = pos[e_src[m0], d]
        PT_full[3 + d, gcol[m0]] = pos[e_dst[m0], d]
        PT_full[64 + d, gcol[m1]] = pos[e_src[m1], d]
        PT_full[67 + d, gcol[m1]] = pos[e_dst[m1], d]

    total = N_BUCKETS * S
    XS_full = np.zeros((total, IN_CH), dtype=np.float32)
    XS_full[slot] = x_flat[e_src]

    # one-hot dst matrices, [bucket, partition(slot%128), chunk, node] bf16
    OH_full = np.zeros((N_BUCKETS, 128, K, 128), dtype=BF16)
    b_g = (slot // S).astype(np.int64)
    p_g = (sl % 128).astype(np.int64)
    c_g = (sl // 128).astype(np.int64)
    n_g = (e_dst - (sb.astype(np.int64) << 7)).astype(np.int64)
    OH_full[b_g, p_g, c_g, n_g] = BF16(1.0)

    # exact bias aggregation on host: xs_agg[n, i] = sum_{e: dst=n} x[src, i]
    xs_agg = np.zeros((N_POINTS, IN_CH), dtype=np.float32)
    for i in range(IN_CH):
        xs_agg[:, i] = np.bincount(dst, weights=x_flat[src, i],
                                   minlength=N_POINTS).astype(np.float32)

    per_core = []
    core_slots = B_PER_CORE * S
    for ci in range(N_CORES):
        sl_ = slice(ci * core_slots, (ci + 1) * core_slots)
        ptc = np.ascontiguousarray(
            PT_full[:, ci * B_PER_CORE * CB:(ci + 1) * B_PER_CORE * CB]
        ).astype(BF16)
        xsc = XS_full[sl_].reshape(B_PER_CORE, K, 128, IN_CH)
        xsc = np.ascontiguousarray(xsc.transpose(0, 2, 1, 3)).reshape(
            B_PER_CORE, 128, K * IN_CH).astype(BF16)
        ohc = np.ascontiguousarray(
            OH_full[ci * B_PER_CORE:(ci + 1) * B_PER_CORE]).reshape(
            B_PER_CORE, 128, K * 128)
        per_core.append({"PT": ptc, "XSB": xsc, "OH": ohc})

    # weights (shared across cores); contraction rows zero-padded to 64
    W1a = np.asarray(W1, dtype=np.float32)                          # [6, 64]
    W1s = np.zeros((128, HID), dtype=np.float32)
    W1s[0:6] = W1a
    W1s[64:70] = W1a
    b1a = np.asarray(b1, dtype=np.float32).reshape(HID, 1)
    B1s = np.concatenate([b1a, b1a], axis=0)
    Wha = np.asarray(Wh, dtype=np.float32)
    Wh2 = np.concatenate([Wha, Wha], axis=0)
    bha = np.asarray(bh, dtype=np.float32).reshape(HID, 1)
    Bh2 = np.concatenate([bha, bha], axis=0)
    WoP = np.asarray(Wo, dtype=np.float32).reshape(HID, IN_CH, OUT_CH)
    WoP = np.ascontiguousarray(WoP.transpose(0, 2, 1)).reshape(HID, 256)
    Wo2 = np.concatenate([WoP, WoP], axis=0)
    shared = {"W1S": W1s.astype(BF16), "B1S": B1s,
              "WH2": Wh2.astype(BF16), "BH2": Bh2, "WO2": Wo2.astype(BF16)}
    for m in per_core:
        m.update(shared)
    return K, per_core, xs_agg


def kernel(**inputs):
    from concourse import bass_utils

    K, in_maps, xs_agg = _host_prep(
        inputs["x"], inputs["pos"], inputs["edge_index"],
        inputs["W1"], inputs["b1"], inputs["Wh"], inputs["bh"],
        inputs["Wo"], inputs["bo"])

    if K not in _PROGRAM_CACHE:
        _PROGRAM_CACHE[K] = _build_program(K)
    nc = _PROGRAM_CACHE[K]

    res = bass_utils.run_bass_kernel_spmd(nc, in_maps,
                                          core_ids=list(range(N_CORES)))
    bo_a = np.asarray(inputs["bo"], dtype=np.float32).reshape(IN_CH, OUT_CH)
    outs = []
    for r in res.results:
        arr = np.asarray(r["OUT"], dtype=np.float32).reshape(
            128, B_PER_CORE, 2, 16, 16)
        msum = arr.sum(axis=(2, 4))                 # [128, 49, 16]
        outs.append(np.ascontiguousarray(msum.transpose(1, 0, 2)))  # [49,128,16]
    full = np.concatenate(outs, axis=0).reshape(N_PAD, OUT_CH)
    out = full[:N_POINTS] + xs_agg @ bo_a
    return np.ascontiguousarray(out.reshape(1, N_POINTS, OUT_CH).astype(np.float32))


# revision 16
# speedup vs baseline: 1.0444x; 1.0444x over previous
"""Trainium2 Bass kernel for IntegralTransform GNN message passing.

Strategy (dst-sharded, 8 cores), v4:
  - Node space padded to 50176 = 8 * 49 * 128. Core c owns nodes
    [c*6272, (c+1)*6272) = 49 buckets of 128 nodes.
  - Host bins edges by dst bucket (stable sort), gathers pos[src]|pos[dst]
    and x[src], pads to K chunks of 128 edges per bucket, and ALSO
    precomputes the per-chunk one-hot dst matrices (bf16) so no engine
    spends time on iota/compare — they stream in over the idle DMA path.
  - PE array packing: L1/L2 run as two concurrent 64x64 tiles
    (T0 = partitions 0-63, T10 = 64-127; contraction zero-padded to 64),
    L3 and the one-hot scatter run as two concurrent 64x128 row tiles
    (T0/T8), which also hides LDWEIGHTS behind the other tile's matmul.
  - Per chunk, L3 makes hp = h2 @ Wo edge-major ([128e, 256(o,i)]); DVE
    multiplies hp (PSUM, 4 chunks per instruction) by xs broadcast along
    o into bf16 prod tiles; one-hot matmuls scatter prod into two
    node-major PSUM accumulators (one per edge half).
  - Bucket drain: DVE group-reduces both accs over i -> [128, 2, 16].
  - Host sums the halves, adds the exact bias term via a bincount
    xs-aggregation: out = msum + xs_agg @ bo. No collectives.
"""

import numpy as np
import ml_dtypes

N_POINTS = 50000
N_PAD = 50176          # 8 * 49 * 128
N_CORES = 8
BUCKET = 128           # nodes per bucket
B_PER_CORE = 49
N_BUCKETS = N_PAD // BUCKET   # 392
CORE_NODES = B_PER_CORE * BUCKET  # 6272
IN_CH = 16
OUT_CH = 16
HID = 64
POS = 3

BF16 = ml_dtypes.bfloat16

_PROGRAM_CACHE = {}


def _geom(K):
    """Pair-tile geometry for S = K*128 edge slots per bucket."""
    S = K * 128
    n_full = S // 1024           # full pair tiles (1024 slots as [128, 512])
    tail = S - n_full * 1024     # leftover slots, multiple of 128, < 1024
    CB = n_full * 512 + tail     # pos-enc columns per bucket (tail unstacked)
    return S, n_full, tail, CB


def _build_program(K):
    import concourse.bacc as bacc
    import concourse.tile as tile
    import concourse.mybir as mybir

    f32 = mybir.dt.float32
    bf16 = mybir.dt.bfloat16
    S, n_full, tail, CB = _geom(K)

    nc = bacc.Bacc("TRN2", target_bir_lowering=False, debug=False)

    PT = nc.dram_tensor("PT", [128, B_PER_CORE * CB], bf16, kind="ExternalInput")
    XSB = nc.dram_tensor("XSB", [B_PER_CORE, 128, K * 16], bf16, kind="ExternalInput")
    OH = nc.dram_tensor("OH", [B_PER_CORE, 128, K * 128], bf16, kind="ExternalInput")
    W1S = nc.dram_tensor("W1S", [128, HID], bf16, kind="ExternalInput")
    B1S = nc.dram_tensor("B1S", [128, 1], f32, kind="ExternalInput")
    WH2 = nc.dram_tensor("WH2", [128, HID], bf16, kind="ExternalInput")
    BH2 = nc.dram_tensor("BH2", [128, 1], f32, kind="ExternalInput")
    WO2 = nc.dram_tensor("WO2", [128, 256], bf16, kind="ExternalInput")
    OUT = nc.dram_tensor("OUT", [128, B_PER_CORE * 32], f32, kind="ExternalOutput")

    Gelu = mybir.ActivationFunctionType.Gelu
    Copy = mybir.ActivationFunctionType.Copy
    MUL = mybir.AluOpType.mult
    ADD = mybir.AluOpType.add
    X = mybir.AxisListType.X

    # chunk -> (partition row base, column range) inside its bucket's h2 tiles
    def chunk_loc(c):
        s0 = c * 128
        if s0 < n_full * 1024:
            t = s0 // 1024
            w = s0 % 1024
            if w < 512:
                return 0, t * 512 + w
            return 64, t * 512 + (w - 512)
        return 0, n_full * 512 + (s0 - n_full * 1024)

    batches = []
    c = 0
    while c < K:
        n = min(4, K - c)
        batches.append((c, n))
        c += n

    with tile.TileContext(nc) as tc:
        with tc.tile_pool(name="const", bufs=1) as cp, \
             tc.tile_pool(name="io", bufs=3) as io, \
             tc.tile_pool(name="h1p", bufs=3) as h1p, \
             tc.tile_pool(name="h2p", bufs=6) as h2p, \
             tc.tile_pool(name="wk", bufs=4) as wk, \
             tc.tile_pool(name="psL", bufs=2, space="PSUM") as psL, \
             tc.tile_pool(name="psH", bufs=2, space="PSUM") as psH, \
             tc.tile_pool(name="psAcc", bufs=1, space="PSUM") as psAcc:

            # --- constants ---
            w1s_t = cp.tile([128, HID], bf16)
            nc.sync.dma_start(out=w1s_t[:], in_=W1S[:])
            b1s_t = cp.tile([128, 1], f32)
            nc.sync.dma_start(out=b1s_t[:], in_=B1S[:])
            wh2_t = cp.tile([128, HID], bf16)
            nc.sync.dma_start(out=wh2_t[:], in_=WH2[:])
            bh2_t = cp.tile([128, 1], f32)
            nc.sync.dma_start(out=bh2_t[:], in_=BH2[:])
            wo2_t = cp.tile([128, 256], bf16)
            nc.sync.dma_start(out=wo2_t[:], in_=WO2[:])
            fin_sb = cp.tile([128, B_PER_CORE, 32], f32)

            h2_tiles = {}

            def stage1(b):
                """DMA in + L1/L2 MLP (64x64 packed pairs)."""
                pt_t = io.tile([128, CB], bf16, tag="pt")
                nc.sync.dma_start(out=pt_t[:], in_=PT[:, b * CB:(b + 1) * CB])
                xsb_t = io.tile([128, K, 16], bf16, tag="xsb")
                nc.sync.dma_start(out=xsb_t[:], in_=XSB[b])
                oh_t = io.tile([128, K, 128], bf16, tag="oh")
                nc.sync.dma_start(out=oh_t[:], in_=OH[b])

                tiles = []
                for t in range(n_full):
                    c0 = t * 512
                    p1 = psL.tile([128, 512], f32, tag="psl")
                    nc.tensor.matmul(p1[0:64, :], lhsT=w1s_t[0:64, :],
                                     rhs=pt_t[0:64, c0:c0 + 512],
                                     start=True, stop=True)
                    nc.tensor.matmul(p1[64:128, :], lhsT=w1s_t[64:128, :],
                                     rhs=pt_t[64:128, c0:c0 + 512],
                                     start=True, stop=True)
                    h1_t = h1p.tile([128, 512], bf16, tag="h1")
                    nc.scalar.activation(h1_t[:], p1[:], Gelu,
                                         bias=b1s_t[:], scale=1.0)
                    p2 = psL.tile([128, 512], f32, tag="psl")
                    nc.tensor.matmul(p2[0:64, :], lhsT=wh2_t[0:64, :],
                                     rhs=h1_t[0:64, :], start=True, stop=True)
                    nc.tensor.matmul(p2[64:128, :], lhsT=wh2_t[64:128, :],
                                     rhs=h1_t[64:128, :], start=True, stop=True)
                    h2_t = h2p.tile([128, 512], bf16, tag="h2")
                    nc.scalar.activation(h2_t[:], p2[:], Gelu,
                                         bias=bh2_t[:], scale=1.0)
                    tiles.append(h2_t)
                if tail:
                    h2tl = h2p.tile([64, tail], bf16, tag="h2t")
                    for w0 in range(0, tail, 512):
                        w = min(512, tail - w0)
                        c0 = n_full * 512 + w0
                        p1 = psL.tile([128, 512], f32, tag="psl")
                        nc.tensor.matmul(p1[0:64, 0:w], lhsT=w1s_t[0:64, :],
                                         rhs=pt_t[0:64, c0:c0 + w],
                                         start=True, stop=True)
                        h1tl = h1p.tile([64, 512], bf16, tag="h1t")
                        nc.scalar.activation(h1tl[:, 0:w], p1[0:64, 0:w], Gelu,
                                             bias=b1s_t[0:64], scale=1.0)
                        p2 = psL.tile([128, 512], f32, tag="psl")
                        nc.tensor.matmul(p2[0:64, 0:w], lhsT=wh2_t[0:64, :],
                                         rhs=h1tl[0:64, 0:w], start=True, stop=True)
                        nc.scalar.activation(h2tl[:, w0:w0 + w], p2[0:64, 0:w],
                                             Gelu, bias=bh2_t[0:64], scale=1.0)
                    tiles.append(h2tl)
                h2_tiles[b] = (tiles, xsb_t, oh_t)

            def emit_l3(c, hp_t, j, tiles):
                row, col = chunk_loc(c)
                if row == 0 and col >= n_full * 512:
                    src = tiles[n_full]   # tail tile
                    lhsT = src[0:64, col - n_full * 512:
                               col - n_full * 512 + 128]
                    rhs = wo2_t[0:64, :]
                else:
                    src = tiles[col // 512]
                    cc = col % 512
                    lhsT = src[row:row + 64, cc:cc + 128]
                    rhs = wo2_t[row:row + 64, :]
                nc.tensor.matmul(hp_t[:, j * 256:(j + 1) * 256],
                                 lhsT=lhsT, rhs=rhs, start=True, stop=True)

            def stage2(b):
                """L3 + einsum + scatter (64x128 row tiles T0/T8)."""
                tiles, xsb_t, oh_t = h2_tiles.pop(b)

                acc2 = psAcc.tile([128, 2, 512], f32, tag="acc")
                pend = []
                bi = 0
                while bi < len(batches):
                    # Process two consecutive batches together, interleaving
                    # their L3 matmuls: adjacent batches cover the two PE
                    # half-regions (T0/T8 row tiles) and sit in different
                    # PSUM banks, so interleaved matmuls run concurrently.
                    grp = batches[bi:bi + 2]
                    bi += len(grp)
                    hps = []
                    for _g in range(len(grp)):
                        hp_t = psH.tile([128, 4 * 256], f32, tag="hp",
                                        name=f"hp_{bi}_{_g}")
                        hps.append(hp_t)
                    maxn = max(n for (_, n) in grp)
                    for j in range(maxn):
                        for g, (c0, n) in enumerate(grp):
                            if j < n:
                                emit_l3(c0 + j, hps[g], j, tiles)
                    for g, (c0, n) in enumerate(grp):
                        # prod tile [128, c, 16, 16] bf16 = hp * xs(bcast o)
                        scat4 = wk.tile([128, 4, 16, 16], bf16, tag="scat")
                        nc.vector.tensor_tensor(
                            out=scat4[:, 0:n, :, :],
                            in0=hps[g][:, 0:n * 256].rearrange(
                                "p (c o i) -> p c o i", c=n, o=16, i=16),
                            in1=xsb_t[:, c0:c0 + n, :].unsqueeze(2)
                                .to_broadcast([128, n, 16, 16]),
                            op=MUL)
                        pend.append((scat4, c0, n))
                        if len(pend) == 3:
                            _scatter(pend.pop(0), oh_t, acc2)
                while pend:
                    _scatter(pend.pop(0), oh_t, acc2)

                # bucket drain: [128, 2, 16] group-reduced msum halves
                nc.vector.tensor_reduce(
                    out=fin_sb[:, b, 0:32],
                    in_=acc2[:, :, 0:256].rearrange(
                        "p g (o i) -> p g o i", o=16, i=16),
                    axis=X, op=ADD)

            def _scatter(item, oh_t, acc2):
                scat4, c0, n = item
                for j in range(n):
                    c = c0 + j
                    st = (c == 0)
                    sp = (c == K - 1)
                    nc.tensor.matmul(acc2[:, 0, 0:256],
                                     lhsT=oh_t[0:64, c, :],
                                     rhs=scat4[0:64, j, :, :],
                                     start=st, stop=sp)
                    nc.tensor.matmul(acc2[:, 1, 0:256],
                                     lhsT=oh_t[64:128, c, :],
                                     rhs=scat4[64:128, j, :, :],
                                     start=st, stop=sp)

            stage1(0)
            for b in range(B_PER_CORE):
                if b + 1 < B_PER_CORE:
                    stage1(b + 1)
                stage2(b)

            nc.sync.dma_start(out=OUT[:], in_=fin_sb[:])

    nc.compile()
    return nc


def _host_prep(x, pos, edge_index, W1, b1, Wh, bh, Wo, bo):
    """Bin edges by dst bucket, gather, pad; build per-core input maps."""
    x_flat = np.ascontiguousarray(x.reshape(-1, IN_CH).astype(np.float32))
    pos = np.ascontiguousarray(pos.astype(np.float32))
    src = np.asarray(edge_index[0], dtype=np.int64)
    dst = np.asarray(edge_index[1], dtype=np.int64)
    E = src.shape[0]

    bucket = (dst >> 7).astype(np.int32)          # 0..390
    order = np.argsort(bucket, kind="stable")
    sb = bucket[order]
    cnt = np.bincount(bucket, minlength=N_BUCKETS)
    K = int(np.max((cnt + 127) // 128))
    S, n_full, tail, CB = _geom(K)

    starts = np.zeros(N_BUCKETS, dtype=np.int64)
    starts[1:] = np.cumsum(cnt)[:-1]
    rank = np.arange(E, dtype=np.int64) - starts[sb]
    slot = sb.astype(np.int64) * S + rank          # global slot id

    e_src = src[order]
    e_dst = dst[order]

    # pos-enc stream in pair-stacked layout
    sl = slot % S
    in_full = sl < n_full * 1024
    w = sl % 1024
    col_full = (sl // 1024) * 512 + np.where(w < 512, w, w - 512)
    grp_full = (w >= 512).astype(np.int64)
    col_tail = n_full * 512 + (sl - n_full * 1024)
    colb = np.where(in_full, col_full, col_tail)
    grp = np.where(in_full, grp_full, 0)
    gcol = (slot // S) * CB + colb

    PT_full = np.zeros((128, N_BUCKETS * CB), dtype=np.float32)
    m0 = grp == 0
    m1 = ~m0
    for d in range(POS):
        PT_full[0 + d, gcol[m0]] = pos[e_src[m0], d]
        PT_full[3 + d, gcol[m0]] = pos[e_dst[m0], d]
        PT_full[64 + d, gcol[m1]] = pos[e_src[m1], d]
        PT_full[67 + d, gcol[m1]] = pos[e_dst[m1], d]

    total = N_BUCKETS * S
    XS_full = np.zeros((total, IN_CH), dtype=np.float32)
    XS_full[slot] = x_flat[e_src]

    # one-hot dst matrices, [bucket, partition(slot%128), chunk, node] bf16
    OH_full = np.zeros((N_BUCKETS, 128, K, 128), dtype=BF16)
    b_g = (slot // S).astype(np.int64)
    p_g = (sl % 128).astype(np.int64)
    c_g = (sl // 128).astype(np.int64)
    n_g = (e_dst - (sb.astype(np.int64) << 7)).astype(np.int64)
    OH_full[b_g, p_g, c_g, n_g] = BF16(1.0)

    # exact bias aggregation on host: xs_agg[n, i] = sum_{e: dst=n} x[src, i]
    xs_agg = np.zeros((N_POINTS, IN_CH), dtype=np.float32)
    for i in range(IN_CH):
        xs_agg[:, i] = np.bincount(dst, weights=x_flat[src, i],
                                   minlength=N_POINTS).astype(np.float32)

    per_core = []
    core_slots = B_PER_CORE * S
    for ci in range(N_CORES):
        sl_ = slice(ci * core_slots, (ci + 1) * core_slots)
        ptc = np.ascontiguousarray(
            PT_full[:, ci * B_PER_CORE * CB:(ci + 1) * B_PER_CORE * CB]
        ).astype(BF16)
        xsc = XS_full[sl_].reshape(B_PER_CORE, K, 128, IN_CH)
        xsc = np.ascontiguousarray(xsc.transpose(0, 2, 1, 3)).reshape(
            B_PER_CORE, 128, K * IN_CH).astype(BF16)
        ohc = np.ascontiguousarray(
            OH_full[ci * B_PER_CORE:(ci + 1) * B_PER_CORE]).reshape(
            B_PER_CORE, 128, K * 128)
        per_core.append({"PT": ptc, "XSB": xsc, "OH": ohc})

    # weights (shared across cores); contraction rows zero-padded to 64
    W1a = np.asarray(W1, dtype=np.float32)                          # [6, 64]
    W1s = np.zeros((128, HID), dtype=np.float32)
    W1s[0:6] = W1a
    W1s[64:70] = W1a
    b1a = np.asarray(b1, dtype=np.float32).reshape(HID, 1)
    B1s = np.concatenate([b1a, b1a], axis=0)
    Wha = np.asarray(Wh, dtype=np.float32)
    Wh2 = np.concatenate([Wha, Wha], axis=0)
    bha = np.asarray(bh, dtype=np.float32).reshape(HID, 1)
    Bh2 = np.concatenate([bha, bha], axis=0)
    WoP = np.asarray(Wo, dtype=np.float32).reshape(HID, IN_CH, OUT_CH)
    WoP = np.ascontiguousarray(WoP.transpose(0, 2, 1)).reshape(HID, 256)
    Wo2 = np.concatenate([WoP, WoP], axis=0)
    shared = {"W1S": W1s.astype(BF16), "B1S": B1s,
              "WH2": Wh2.astype(BF16), "BH2": Bh2, "WO2": Wo2.astype(BF16)}
    for m in per_core:
        m.update(shared)
    return K, per_core, xs_agg


def kernel(**inputs):
    from concourse import bass_utils

    K, in_maps, xs_agg = _host_prep(
        inputs["x"], inputs["pos"], inputs["edge_index"],
        inputs["W1"], inputs["b1"], inputs["Wh"], inputs["bh"],
        inputs["Wo"], inputs["bo"])

    if K not in _PROGRAM_CACHE:
        _PROGRAM_CACHE[K] = _build_program(K)
    nc = _PROGRAM_CACHE[K]

    res = bass_utils.run_bass_kernel_spmd(nc, in_maps,
                                          core_ids=list(range(N_CORES)))
    bo_a = np.asarray(inputs["bo"], dtype=np.float32).reshape(IN_CH, OUT_CH)
    outs = []
    for r in res.results:
        arr = r["OUT"].reshape(128, B_PER_CORE, 32)
        msum = arr[:, :, 0:16] + arr[:, :, 16:32]
        outs.append(np.ascontiguousarray(msum.transpose(1, 0, 2)))  # [49,128,16]
    full = np.concatenate(outs, axis=0).reshape(N_PAD, OUT_CH)
    out = full[:N_POINTS] + xs_agg @ bo_a
    return np.ascontiguousarray(out.reshape(1, N_POINTS, OUT_CH).astype(np.float32))


# revision 17
# speedup vs baseline: 1.0545x; 1.0097x over previous
"""Trainium2 Bass kernel for IntegralTransform GNN message passing.

Strategy (dst-sharded, 8 cores), v4:
  - Node space padded to 50176 = 8 * 49 * 128. Core c owns nodes
    [c*6272, (c+1)*6272) = 49 buckets of 128 nodes.
  - Host bins edges by dst bucket (stable sort), gathers pos[src]|pos[dst]
    and x[src], pads to K chunks of 128 edges per bucket, and ALSO
    precomputes the per-chunk one-hot dst matrices (bf16) so no engine
    spends time on iota/compare — they stream in over the idle DMA path.
  - PE array packing: L1/L2 run as two concurrent 64x64 tiles
    (T0 = partitions 0-63, T10 = 64-127; contraction zero-padded to 64),
    L3 and the one-hot scatter run as two concurrent 64x128 row tiles
    (T0/T8), which also hides LDWEIGHTS behind the other tile's matmul.
  - Per chunk, L3 makes hp = h2 @ Wo edge-major ([128e, 256(o,i)]); DVE
    multiplies hp (PSUM, 4 chunks per instruction) by xs broadcast along
    o into bf16 prod tiles; one-hot matmuls scatter prod into two
    node-major PSUM accumulators (one per edge half).
  - Bucket drain: DVE group-reduces both accs over i -> [128, 2, 16].
  - Host sums the halves, adds the exact bias term via a bincount
    xs-aggregation: out = msum + xs_agg @ bo. No collectives.
"""

import numpy as np
import ml_dtypes

N_POINTS = 50000
N_PAD = 50176          # 8 * 49 * 128
N_CORES = 8
BUCKET = 128           # nodes per bucket
B_PER_CORE = 49
N_BUCKETS = N_PAD // BUCKET   # 392
CORE_NODES = B_PER_CORE * BUCKET  # 6272
IN_CH = 16
OUT_CH = 16
HID = 64
POS = 3

BF16 = ml_dtypes.bfloat16

_PROGRAM_CACHE = {}


def _geom(K):
    """Pair-tile geometry for S = K*128 edge slots per bucket."""
    S = K * 128
    n_full = S // 1024           # full pair tiles (1024 slots as [128, 512])
    tail = S - n_full * 1024     # leftover slots, multiple of 128, < 1024
    CB = n_full * 512 + tail     # pos-enc columns per bucket (tail unstacked)
    return S, n_full, tail, CB


def _build_program(K):
    import concourse.bacc as bacc
    import concourse.tile as tile
    import concourse.mybir as mybir

    f32 = mybir.dt.float32
    bf16 = mybir.dt.bfloat16
    S, n_full, tail, CB = _geom(K)

    nc = bacc.Bacc("TRN2", target_bir_lowering=False, debug=False)

    PT = nc.dram_tensor("PT", [128, B_PER_CORE * CB], bf16, kind="ExternalInput")
    XSB = nc.dram_tensor("XSB", [B_PER_CORE, 128, K * 16], bf16, kind="ExternalInput")
    OH = nc.dram_tensor("OH", [B_PER_CORE, 128, K * 128], bf16, kind="ExternalInput")
    W1S = nc.dram_tensor("W1S", [128, HID], bf16, kind="ExternalInput")
    B1S = nc.dram_tensor("B1S", [128, 1], f32, kind="ExternalInput")
    WH2 = nc.dram_tensor("WH2", [128, HID], bf16, kind="ExternalInput")
    BH2 = nc.dram_tensor("BH2", [128, 1], f32, kind="ExternalInput")
    WO2 = nc.dram_tensor("WO2", [128, 256], bf16, kind="ExternalInput")
    OUT = nc.dram_tensor("OUT", [128, B_PER_CORE * 32], f32, kind="ExternalOutput")

    Gelu = mybir.ActivationFunctionType.Gelu
    MUL = mybir.AluOpType.mult
    ADD = mybir.AluOpType.add
    X = mybir.AxisListType.X

    # chunk -> (partition row base, column range) inside its bucket's h2 tiles
    def chunk_loc(c):
        s0 = c * 128
        if s0 < n_full * 1024:
            t = s0 // 1024
            w = s0 % 1024
            if w < 512:
                return 0, t * 512 + w
            return 64, t * 512 + (w - 512)
        return 0, n_full * 512 + (s0 - n_full * 1024)

    batches = []
    c = 0
    while c < K:
        n = min(4, K - c)
        batches.append((c, n))
        c += n

    with tile.TileContext(nc) as tc:
        with tc.tile_pool(name="const", bufs=1) as cp, \
             tc.tile_pool(name="io", bufs=3) as io, \
             tc.tile_pool(name="h1p", bufs=3) as h1p, \
             tc.tile_pool(name="h2p", bufs=6) as h2p, \
             tc.tile_pool(name="wk", bufs=4) as wk, \
             tc.tile_pool(name="psL", bufs=2, space="PSUM") as psL, \
             tc.tile_pool(name="psH", bufs=2, space="PSUM") as psH, \
             tc.tile_pool(name="psAcc", bufs=1, space="PSUM") as psAcc:

            # --- constants ---
            w1s_t = cp.tile([128, HID], bf16)
            nc.sync.dma_start(out=w1s_t[:], in_=W1S[:])
            b1s_t = cp.tile([128, 1], f32)
            nc.sync.dma_start(out=b1s_t[:], in_=B1S[:])
            wh2_t = cp.tile([128, HID], bf16)
            nc.sync.dma_start(out=wh2_t[:], in_=WH2[:])
            bh2_t = cp.tile([128, 1], f32)
            nc.sync.dma_start(out=bh2_t[:], in_=BH2[:])
            wo2_t = cp.tile([128, 256], bf16)
            nc.sync.dma_start(out=wo2_t[:], in_=WO2[:])
            fin_sb = cp.tile([128, B_PER_CORE, 32], f32)

            h2_tiles = {}

            def stage1(b):
                """DMA in + L1/L2 MLP (64x64 packed pairs)."""
                pt_t = io.tile([128, CB], bf16, tag="pt")
                nc.sync.dma_start(out=pt_t[:], in_=PT[:, b * CB:(b + 1) * CB])
                xsb_t = io.tile([128, K, 16], bf16, tag="xsb")
                nc.sync.dma_start(out=xsb_t[:], in_=XSB[b])
                oh_t = io.tile([128, K, 128], bf16, tag="oh")
                nc.sync.dma_start(out=oh_t[:], in_=OH[b])

                tiles = []
                for t in range(n_full):
                    c0 = t * 512
                    p1 = psL.tile([128, 512], f32, tag="psl")
                    nc.tensor.matmul(p1[0:64, :], lhsT=w1s_t[0:64, :],
                                     rhs=pt_t[0:64, c0:c0 + 512],
                                     start=True, stop=True)
                    nc.tensor.matmul(p1[64:128, :], lhsT=w1s_t[64:128, :],
                                     rhs=pt_t[64:128, c0:c0 + 512],
                                     start=True, stop=True)
                    h1_t = h1p.tile([128, 512], bf16, tag="h1")
                    nc.scalar.activation(h1_t[:], p1[:], Gelu,
                                         bias=b1s_t[:], scale=1.0)
                    p2 = psL.tile([128, 512], f32, tag="psl")
                    nc.tensor.matmul(p2[0:64, :], lhsT=wh2_t[0:64, :],
                                     rhs=h1_t[0:64, :], start=True, stop=True)
                    nc.tensor.matmul(p2[64:128, :], lhsT=wh2_t[64:128, :],
                                     rhs=h1_t[64:128, :], start=True, stop=True)
                    h2_t = h2p.tile([128, 512], bf16, tag="h2")
                    nc.scalar.activation(h2_t[:], p2[:], Gelu,
                                         bias=bh2_t[:], scale=1.0)
                    tiles.append(h2_t)
                if tail:
                    h2tl = h2p.tile([64, tail], bf16, tag="h2t")
                    for w0 in range(0, tail, 512):
                        w = min(512, tail - w0)
                        c0 = n_full * 512 + w0
                        p1 = psL.tile([128, 512], f32, tag="psl")
                        nc.tensor.matmul(p1[0:64, 0:w], lhsT=w1s_t[0:64, :],
                                         rhs=pt_t[0:64, c0:c0 + w],
                                         start=True, stop=True)
                        h1tl = h1p.tile([64, 512], bf16, tag="h1t")
                        nc.scalar.activation(h1tl[:, 0:w], p1[0:64, 0:w], Gelu,
                                             bias=b1s_t[0:64], scale=1.0)
                        p2 = psL.tile([128, 512], f32, tag="psl")
                        nc.tensor.matmul(p2[0:64, 0:w], lhsT=wh2_t[0:64, :],
                                         rhs=h1tl[0:64, 0:w], start=True, stop=True)
                        nc.scalar.activation(h2tl[:, w0:w0 + w], p2[0:64, 0:w],
                                             Gelu, bias=bh2_t[0:64], scale=1.0)
                    tiles.append(h2tl)
                h2_tiles[b] = (tiles, xsb_t, oh_t)

            def stage2(b):
                """L3 + einsum + scatter (64x128 row tiles T0/T8)."""
                tiles, xsb_t, oh_t = h2_tiles.pop(b)

                acc2 = psAcc.tile([128, 2, 512], f32, tag="acc")
                pend = []
                for (c0, n) in batches:
                    hp_t = psH.tile([128, 4 * 256], f32, tag="hp")
                    for j in range(n):
                        c = c0 + j
                        row, col = chunk_loc(c)
                        if row == 0 and col >= n_full * 512:
                            src = tiles[n_full]   # tail tile
                            lhsT = src[0:64, col - n_full * 512:
                                       col - n_full * 512 + 128]
                            rhs = wo2_t[0:64, :]
                        else:
                            src = tiles[col // 512]
                            cc = col % 512
                            lhsT = src[row:row + 64, cc:cc + 128]
                            rhs = wo2_t[row:row + 64, :]
                        nc.tensor.matmul(hp_t[:, j * 256:(j + 1) * 256],
                                         lhsT=lhsT, rhs=rhs,
                                         start=True, stop=True)

                    # prod tile [128, c, 16, 16] bf16 = hp * xs(broadcast o)
                    scat4 = wk.tile([128, 4, 16, 16], bf16, tag="scat")
                    nc.vector.tensor_tensor(
                        out=scat4[:, 0:n, :, :],
                        in0=hp_t[:, 0:n * 256].rearrange(
                            "p (c o i) -> p c o i", c=n, o=16, i=16),
                        in1=xsb_t[:, c0:c0 + n, :].unsqueeze(2).to_broadcast(
                            [128, n, 16, 16]),
                        op=MUL)
                    pend.append((scat4, c0, n))

                    if len(pend) == 3:
                        _scatter(pend.pop(0), oh_t, acc2)
                while pend:
                    _scatter(pend.pop(0), oh_t, acc2)

                # bucket drain: [128, 2, 16] group-reduced msum halves
                nc.vector.tensor_reduce(
                    out=fin_sb[:, b, 0:32],
                    in_=acc2[:, :, 0:256].rearrange(
                        "p g (o i) -> p g o i", o=16, i=16),
                    axis=X, op=ADD)

            def _scatter(item, oh_t, acc2):
                scat4, c0, n = item
                for j in range(n):
                    c = c0 + j
                    st = (c == 0)
                    sp = (c == K - 1)
                    nc.tensor.matmul(acc2[:, 0, 0:256],
                                     lhsT=oh_t[0:64, c, :],
                                     rhs=scat4[0:64, j, :, :],
                                     start=st, stop=sp)
                    nc.tensor.matmul(acc2[:, 1, 0:256],
                                     lhsT=oh_t[64:128, c, :],
                                     rhs=scat4[64:128, j, :, :],
                                     start=st, stop=sp)

            stage1(0)
            for b in range(B_PER_CORE):
                if b + 1 < B_PER_CORE:
                    stage1(b + 1)
                stage2(b)

            nc.sync.dma_start(out=OUT[:], in_=fin_sb[:])

    nc.compile()
    return nc


def _host_prep(x, pos, edge_index, W1, b1, Wh, bh, Wo, bo):
    """Bin edges by dst bucket, gather, pad; build per-core input maps."""
    x_flat = np.ascontiguousarray(x.reshape(-1, IN_CH).astype(np.float32))
    pos = np.ascontiguousarray(pos.astype(np.float32))
    src = np.asarray(edge_index[0], dtype=np.int64)
    dst = np.asarray(edge_index[1], dtype=np.int64)
    E = src.shape[0]

    bucket = (dst >> 7).astype(np.int32)          # 0..390
    order = np.argsort(bucket, kind="stable")
    sb = bucket[order]
    cnt = np.bincount(bucket, minlength=N_BUCKETS)
    K = int(np.max((cnt + 127) // 128))
    S, n_full, tail, CB = _geom(K)

    starts = np.zeros(N_BUCKETS, dtype=np.int64)
    starts[1:] = np.cumsum(cnt)[:-1]
    rank = np.arange(E, dtype=np.int64) - starts[sb]
    slot = sb.astype(np.int64) * S + rank          # global slot id

    e_src = src[order]
    e_dst = dst[order]

    # pos-enc stream in pair-stacked layout
    sl = slot % S
    in_full = sl < n_full * 1024
    w = sl % 1024
    col_full = (sl // 1024) * 512 + np.where(w < 512, w, w - 512)
    grp_full = (w >= 512).astype(np.int64)
    col_tail = n_full * 512 + (sl - n_full * 1024)
    colb = np.where(in_full, col_full, col_tail)
    grp = np.where(in_full, grp_full, 0)
    gcol = (slot // S) * CB + colb

    PT_full = np.zeros((128, N_BUCKETS * CB), dtype=np.float32)
    m0 = grp == 0
    m1 = ~m0
    for d in range(POS):
        PT_full[0 + d, gcol[m0]] = pos[e_src[m0], d]
        PT_full[3 + d, gcol[m0]] = pos[e_dst[m0], d]
        PT_full[64 + d, gcol[m1]] = pos[e_src[m1], d]
        PT_full[67 + d, gcol[m1]] = pos[e_dst[m1], d]

    total = N_BUCKETS * S
    XS_full = np.zeros((total, IN_CH), dtype=np.float32)
    XS_full[slot] = x_flat[e_src]

    # one-hot dst matrices, [bucket, partition(slot%128), chunk, node] bf16
    OH_full = np.zeros((N_BUCKETS, 128, K, 128), dtype=BF16)
    b_g = (slot // S).astype(np.int64)
    p_g = (sl % 128).astype(np.int64)
    c_g = (sl // 128).astype(np.int64)
    n_g = (e_dst - (sb.astype(np.int64) << 7)).astype(np.int64)
    OH_full[b_g, p_g, c_g, n_g] = BF16(1.0)

    # exact bias aggregation on host: xs_agg[n, i] = sum_{e: dst=n} x[src, i]
    xs_agg = np.zeros((N_POINTS, IN_CH), dtype=np.float32)
    for i in range(IN_CH):
        xs_agg[:, i] = np.bincount(dst, weights=x_flat[src, i],
                                   minlength=N_POINTS).astype(np.float32)

    per_core = []
    core_slots = B_PER_CORE * S
    for ci in range(N_CORES):
        sl_ = slice(ci * core_slots, (ci + 1) * core_slots)
        ptc = np.ascontiguousarray(
            PT_full[:, ci * B_PER_CORE * CB:(ci + 1) * B_PER_CORE * CB]
        ).astype(BF16)
        xsc = XS_full[sl_].reshape(B_PER_CORE, K, 128, IN_CH)
        xsc = np.ascontiguousarray(xsc.transpose(0, 2, 1, 3)).reshape(
            B_PER_CORE, 128, K * IN_CH).astype(BF16)
        ohc = np.ascontiguousarray(
            OH_full[ci * B_PER_CORE:(ci + 1) * B_PER_CORE]).reshape(
            B_PER_CORE, 128, K * 128)
        per_core.append({"PT": ptc, "XSB": xsc, "OH": ohc})

    # weights (shared across cores); contraction rows zero-padded to 64
    W1a = np.asarray(W1, dtype=np.float32)                          # [6, 64]
    W1s = np.zeros((128, HID), dtype=np.float32)
    W1s[0:6] = W1a
    W1s[64:70] = W1a
    b1a = np.asarray(b1, dtype=np.float32).reshape(HID, 1)
    B1s = np.concatenate([b1a, b1a], axis=0)
    Wha = np.asarray(Wh, dtype=np.float32)
    Wh2 = np.concatenate([Wha, Wha], axis=0)
    bha = np.asarray(bh, dtype=np.float32).reshape(HID, 1)
    Bh2 = np.concatenate([bha, bha], axis=0)
    WoP = np.asarray(Wo, dtype=np.float32).reshape(HID, IN_CH, OUT_CH)
    WoP = np.ascontiguousarray(WoP.transpose(0, 2, 1)).reshape(HID, 256)
    Wo2 = np.concatenate([WoP, WoP], axis=0)
    shared = {"W1S": W1s.astype(BF16), "B1S": B1s,
              "WH2": Wh2.astype(BF16), "BH2": Bh2, "WO2": Wo2.astype(BF16)}
    for m in per_core:
        m.update(shared)
    return K, per_core, xs_agg


def kernel(**inputs):
    from concourse import bass_utils

    K, in_maps, xs_agg = _host_prep(
        inputs["x"], inputs["pos"], inputs["edge_index"],
        inputs["W1"], inputs["b1"], inputs["Wh"], inputs["bh"],
        inputs["Wo"], inputs["bo"])

    if K not in _PROGRAM_CACHE:
        _PROGRAM_CACHE[K] = _build_program(K)
    nc = _PROGRAM_CACHE[K]

    res = bass_utils.run_bass_kernel_spmd(nc, in_maps,
                                          core_ids=list(range(N_CORES)))
    bo_a = np.asarray(inputs["bo"], dtype=np.float32).reshape(IN_CH, OUT_CH)
    outs = []
    for r in res.results:
        arr = r["OUT"].reshape(128, B_PER_CORE, 32)
        msum = arr[:, :, 0:16] + arr[:, :, 16:32]
        outs.append(np.ascontiguousarray(msum.transpose(1, 0, 2)))  # [49,128,16]
    full = np.concatenate(outs, axis=0).reshape(N_PAD, OUT_CH)
    out = full[:N_POINTS] + xs_agg @ bo_a
    return np.ascontiguousarray(out.reshape(1, N_POINTS, OUT_CH).astype(np.float32))
